# revision 1
# baseline (speedup 1.0000x reference)
"""Multi-head attention (B=2, S=2048, H=1024, 16 heads) on 8 trn2 NeuronCores.

Sharding: tensor-parallel over heads — each core owns 2 heads (128 channels of
the QKV projections and 128 input channels of the output projection). Every
core consumes the full (transposed, bf16-cast) activations; partial outputs of
the wo projection are summed on the host.

Device-side dataflow per core (all matmuls bf16 with f32 PSUM accumulation):
  QT[c,s] = (wq_c x^T + bq) : transposed projections, channels on partitions
  KT[c,s] likewise; V[s,c] in natural layout (tokens on partitions)
  scores^T[k,q] = KT_h^T-tile . QT_h  (two heads row-packed on the PE array)
  E = exp(scores/8)  (no max subtraction: scores are ~N(0,1), |s| < ~6)
  O^T[d,q], sums[q] accumulate over key tiles via ones-augmented V (M=65)
  O_norm = O^T * bcast(1/sums); y^T partial = woT_c . O_norm
"""

import os
import threading

import numpy as np
import ml_dtypes

import concourse.bass as bass
import concourse.mybir as mybir
import concourse.tile as tile
from concourse import bacc
from concourse.bass_utils import run_bass_kernel_spmd

BF16 = ml_dtypes.bfloat16
F32 = mybir.dt.float32
BF = mybir.dt.bfloat16

B = 2
S = 2048
H = 1024
NS = B * S          # 4096 tokens
NH_LOCAL = 2        # heads per core
HD = 64             # head dim
CPC = 128           # channels per core
NF = H // 128       # feature chunks
N_CORES = 8

_cache = threading.Lock()
_nc = None

LAST_RESULT = None  # BassKernelResults of the most recent run (for test.py)


def _build_nc():
    nc = bacc.Bacc(None, target_bir_lowering=False, debug=False)

    xq_d = nc.dram_tensor("xq_t", [H, NS], BF, kind="ExternalInput")
    xk_d = nc.dram_tensor("xk_t", [H, NS], BF, kind="ExternalInput")
    xv_d = nc.dram_tensor("xv_t", [H, NS], BF, kind="ExternalInput")
    wq_d = nc.dram_tensor("wq_t", [H, CPC], BF, kind="ExternalInput")
    wk_d = nc.dram_tensor("wk_t", [H, CPC], BF, kind="ExternalInput")
    wv_d = nc.dram_tensor("wv_t", [H, CPC], BF, kind="ExternalInput")
    bq_d = nc.dram_tensor("bq", [CPC, 1], F32, kind="ExternalInput")
    bk_d = nc.dram_tensor("bk", [CPC, 1], F32, kind="ExternalInput")
    bv_d = nc.dram_tensor("bv", [1, CPC], BF, kind="ExternalInput")
    wo_d = nc.dram_tensor("wo_t", [CPC, H], BF, kind="ExternalInput")
    y_d = nc.dram_tensor("y_t", [H, NS], F32, kind="ExternalOutput")

    xq_ap = xq_d.rearrange("(nf p) s -> nf p s", p=128)
    xk_ap = xk_d.rearrange("(nf p) s -> nf p s", p=128)
    xv_ap = xv_d.rearrange("(nf p) s -> nf p s", p=128)
    y_ap = y_d.rearrange("(no p) s -> no p s", p=128)

    Exp = mybir.ActivationFunctionType.Exp
    Copy = mybir.ActivationFunctionType.Identity

    with tile.TileContext(nc) as tc:
        with (
            tc.tile_pool(name="const", bufs=1) as const,
            tc.tile_pool(name="res", bufs=1) as res,
            tc.tile_pool(name="work", bufs=3) as work,
            tc.tile_pool(name="psum", bufs=2, space="PSUM") as psum,
        ):
            # --- constants / weights ---
            wq_sb = const.tile([128, NF, CPC], BF)
            wk_sb = const.tile([128, NF, CPC], BF)
            wv_sb = const.tile([128, NF, CPC], BF)
            wo_sb = const.tile([128, NF, 128], BF)
            bq_sb = const.tile([128, 1], F32)
            bk_sb = const.tile([128, 1], F32)
            bv_sb = const.tile([1, CPC], BF)
            ones1 = const.tile([1, 128], BF)
            nc.sync.dma_start(wq_sb[:], wq_d.rearrange("(nf p) c -> p nf c", p=128))
            nc.sync.dma_start(wk_sb[:], wk_d.rearrange("(nf p) c -> p nf c", p=128))
            nc.sync.dma_start(wv_sb[:], wv_d.rearrange("(nf p) c -> p nf c", p=128))
            nc.sync.dma_start(wo_sb[:], wo_d.rearrange("p (no c) -> p no c", c=128))
            nc.sync.dma_start(bq_sb[:], bq_d[:])
            nc.sync.dma_start(bk_sb[:], bk_d[:])
            nc.sync.dma_start(bv_sb[:], bv_d[:])
            nc.gpsimd.memset(ones1[:], 1.0)

            # --- residents ---
            QT = res.tile([128, NS], BF)
            KT = res.tile([128, NS], BF)
            V0 = res.tile([128, NS // 128, HD + 1], BF)
            V1 = res.tile([128, NS // 128, HD + 1], BF)
            nc.gpsimd.memset(V0[:, :, HD : HD + 1], 1.0)
            nc.gpsimd.memset(V1[:, :, HD : HD + 1], 1.0)

            # --- projections ---
            with tc.tile_pool(name="xin", bufs=10) as xin:
                for name, x_ap, w_sb, b_sb, out_t in (
                    ("q", xq_ap, wq_sb, bq_sb, QT),
                    ("k", xk_ap, wk_sb, bk_sb, KT),
                ):
                    xt = []
                    for f in range(NF):
                        t = xin.tile([128, NS], BF, tag="xc", name=f"x{name}{f}")
                        nc.sync.dma_start(t[:], x_ap[f])
                        xt.append(t)
                    for sw in range(NS // 512):
                        ps = psum.tile([128, 512], F32, tag="s", name=f"ps{name}{sw}")
                        for f in range(NF):
                            nc.tensor.matmul(
                                ps[:],
                                lhsT=w_sb[:, f, :],
                                rhs=xt[f][:, sw * 512 : (sw + 1) * 512],
                                start=(f == 0),
                                stop=(f == NF - 1),
                            )
                        nc.scalar.activation(
                            out_t[:, sw * 512 : (sw + 1) * 512], ps[:], Copy,
                            bias=b_sb[:],
                        )
                # V (natural layout, tokens on partitions)
                xtv = []
                for f in range(NF):
                    t = xin.tile([128, NS], BF, tag="xc", name=f"xv{f}")
                    nc.sync.dma_start(t[:], xv_ap[f])
                    xtv.append(t)
                for si in range(NS // 128):
                    psv = psum.tile([128, 128], F32, tag="s", name=f"psv{si}")
                    for f in range(NF):
                        nc.tensor.matmul(
                            psv[:],
                            lhsT=xtv[f][:, si * 128 : (si + 1) * 128],
                            rhs=wv_sb[:, f, :],
                            start=(f == 0),
                            stop=False,
                        )
                    nc.tensor.matmul(
                        psv[:], lhsT=ones1[:], rhs=bv_sb[:], start=False, stop=True
                    )
                    nc.vector.tensor_copy(V0[:, si, 0:HD], psv[:, 0:HD])
                    nc.vector.tensor_copy(V1[:, si, 0:HD], psv[:, HD:128])

            # --- attention + output projection ---
            with (
                tc.tile_pool(name="epool", bufs=6) as epool,
                tc.tile_pool(name="npool", bufs=2) as npool,
                tc.tile_pool(name="ypool", bufs=3) as ypool,
                tc.tile_pool(name="opsum", bufs=2, space="PSUM") as opsum,
            ):
                for b in range(B):
                    for qw in range(2):
                        q0 = b * S + qw * 1024
                        po0 = opsum.tile([65, 1024], F32, tag="o", name=f"po0_{b}{qw}")
                        po1 = opsum.tile([65, 1024], F32, tag="o", name=f"po1_{b}{qw}")
                        for k2t in range(S // 128):
                            si = b * 16 + k2t
                            ks = si * 128
                            ps0 = psum.tile([128, 1024], F32, tag="s",
                                            name=f"ps0_{b}{qw}{k2t}")
                            ps1 = psum.tile([128, 1024], F32, tag="s",
                                            name=f"ps1_{b}{qw}{k2t}")
                            for hf in range(2):
                                qs = q0 + hf * 512
                                fs = slice(hf * 512, (hf + 1) * 512)
                                nc.tensor.matmul(
                                    ps0[:, fs],
                                    lhsT=KT[0:64, ks : ks + 128],
                                    rhs=QT[0:64, qs : qs + 512],
                                    tile_position=(0, 0),
                                )
                                nc.tensor.matmul(
                                    ps1[:, fs],
                                    lhsT=KT[64:128, ks : ks + 128],
                                    rhs=QT[64:128, qs : qs + 512],
                                    tile_position=(64, 0),
                                )
                            e0 = epool.tile([128, 1024], BF, tag="e",
                                            name=f"e0_{b}{qw}{k2t}")
                            e1 = epool.tile([128, 1024], BF, tag="e",
                                            name=f"e1_{b}{qw}{k2t}")
                            nc.scalar.activation(e0[:], ps0[:], Exp, scale=0.125)
                            nc.scalar.activation(e1[:], ps1[:], Exp, scale=0.125)
                            for hf in range(2):
                                fs = slice(hf * 512, (hf + 1) * 512)
                                nc.tensor.matmul(
                                    po0[:, fs], lhsT=V0[:, si, :], rhs=e0[:, fs],
                                    start=(k2t == 0), stop=(k2t == 15),
                                )
                                nc.tensor.matmul(
                                    po1[:, fs], lhsT=V1[:, si, :], rhs=e1[:, fs],
                                    start=(k2t == 0), stop=(k2t == 15),
                                )
                        # normalize: On[hd, q] = O^T[hd, q] / sums[q]
                        # lane-aligned reciprocal (row 64 -> row 64); gpsimd
                        # broadcast handles the partition shift afterwards
                        r0 = npool.tile([65, 1024], F32, tag="r0", name=f"r0_{b}{qw}")
                        r1 = npool.tile([65, 1024], F32, tag="r1", name=f"r1_{b}{qw}")
                        nc.vector.reciprocal(r0[64:65, :], po0[64:65, :])
                        nc.vector.reciprocal(r1[64:65, :], po1[64:65, :])
                        # partition_broadcast only reads base-partition-0 APs;
                        # DMA shifts the row down first
                        rs0 = npool.tile([1, 1024], F32, tag="rs0", name=f"rs0_{b}{qw}")
                        rs1 = npool.tile([1, 1024], F32, tag="rs1", name=f"rs1_{b}{qw}")
                        nc.scalar.dma_start(rs0[:], r0[64:65, :])
                        nc.scalar.dma_start(rs1[:], r1[64:65, :])
                        rb0 = npool.tile([64, 1024], F32, tag="rb0", name=f"rb0_{b}{qw}")
                        rb1 = npool.tile([64, 1024], F32, tag="rb1", name=f"rb1_{b}{qw}")
                        nc.gpsimd.partition_broadcast(rb0[:], rs0[:])
                        nc.gpsimd.partition_broadcast(rb1[:], rs1[:])
                        on = npool.tile([128, 1024], BF, tag="on", name=f"on_{b}{qw}")
                        on1 = npool.tile([64, 1024], BF, tag="on1", name=f"on1_{b}{qw}")
                        nc.vector.tensor_mul(on[0:64, :], po0[0:64, :], rb0[:])
                        nc.vector.tensor_mul(on1[:], po1[0:64, :], rb1[:])
                        nc.scalar.dma_start(on[64:128, :], on1[:])
                        for oc in range(NF):
                            for hf in range(2):
                                fs = slice(hf * 512, (hf + 1) * 512)
                                py = psum.tile([128, 512], F32, tag="s",
                                               name=f"py_{b}{qw}{oc}{hf}")
                                nc.tensor.matmul(
                                    py[:], lhsT=wo_sb[:, oc, :], rhs=on[:, fs]
                                )
                                ysb = ypool.tile([128, 512], F32, tag="y",
                                                 name=f"y_{b}{qw}{oc}{hf}")
                                nc.vector.tensor_copy(ysb[:], py[:])
                                nc.sync.dma_start(
                                    y_ap[oc, :, q0 + hf * 512 : q0 + (hf + 1) * 512],
                                    ysb[:],
                                )
    nc.compile()
    return nc


def _get_nc():
    global _nc
    with _cache:
        if _nc is None:
            _nc = _build_nc()
        return _nc


def kernel(q, k, v, wq_w, wq_b, wk_w, wk_b, wv_w, wv_b, wo_w, wo_b):
    global LAST_RESULT
    nc = _get_nc()

    def xT(a):
        return np.ascontiguousarray(np.asarray(a).reshape(NS, H).astype(BF16).T)

    xq_t, xk_t, xv_t = xT(q), xT(k), xT(v)
    wq_w = np.asarray(wq_w, dtype=np.float32)
    wk_w = np.asarray(wk_w, dtype=np.float32)
    wv_w = np.asarray(wv_w, dtype=np.float32)
    wo_w = np.asarray(wo_w, dtype=np.float32)

    in_maps = []
    for c in range(N_CORES):
        cs = slice(c * CPC, (c + 1) * CPC)
        in_maps.append({
            "xq_t": xq_t,
            "xk_t": xk_t,
            "xv_t": xv_t,
            "wq_t": np.ascontiguousarray(wq_w[cs, :].astype(BF16).T),
            "wk_t": np.ascontiguousarray(wk_w[cs, :].astype(BF16).T),
            "wv_t": np.ascontiguousarray(wv_w[cs, :].astype(BF16).T),
            "bq": np.asarray(wq_b, np.float32)[cs].reshape(CPC, 1),
            "bk": np.asarray(wk_b, np.float32)[cs].reshape(CPC, 1),
            "bv": np.asarray(wv_b, np.float32)[cs].astype(BF16).reshape(1, CPC),
            "wo_t": np.ascontiguousarray(wo_w[:, cs].astype(BF16).T),
        })

    res = run_bass_kernel_spmd(
        nc, in_maps, core_ids=list(range(N_CORES)),
        trace=bool(int(os.environ.get("MHA_TRACE", "0"))),
    )
    LAST_RESULT = res

    y = res.results[0]["y_t"].astype(np.float64)
    for c in range(1, N_CORES):
        y += res.results[c]["y_t"]
    y = y.T + np.asarray(wo_b, np.float64)[None, :]
    return y.reshape(B, S, H).astype(np.float32)



# revision 11
# speedup vs baseline: 1.8382x; 1.8382x over previous
"""Multi-head attention (B=2, S=2048, H=1024, 16 heads) on 8 trn2 NeuronCores.

Sharding: 2-way batch x 4-way head tensor parallel. Core c owns batch c//4 and
heads 4*(c%4) .. 4*(c%4)+4 (256 channels of the QKV projections, 256 input
channels of the output projection). Each core consumes its batch's activations
(transposed, bf16) and returns a bf16 partial of the wo projection; the host
sums the 4 partials per batch and adds the bias.

Device dataflow per core (matmuls bf16, f32 PSUM):
  QT/KT[c,s]: transposed projections per head-pair hp (c = 2 heads x 64 dims)
  V[s, h, si, d+1]: natural layout, ones-augmented col for softmax sums
  per (qs, hp):  ps[k,2x512q] = KT_hp^T . QT_hp   (two heads via tile rows)
                 e = exp(ps/8)  (one ACT op per key-block, both heads)
                 po_h[65, 512] += [V_h|1]^T e_h   (accumulate over key blocks)
  norm: DMA po rows out of PSUM early, 1/sums via reciprocal_approx_fast,
        gpsimd partition-broadcast, DVE mul -> ON_hp[128, 512] bf16
  wo:   py[oc,512q] = sum_hp woT_hp,oc . ON_hp ; bf16 -> y_t
"""

import os
import threading

import numpy as np
import ml_dtypes

import concourse.bass as bass
import concourse.mybir as mybir
import concourse.tile as tile
from concourse import bacc
from concourse.bass_utils import run_bass_kernel_spmd

BF16 = ml_dtypes.bfloat16
F32 = mybir.dt.float32
BF = mybir.dt.bfloat16

B = 2
S = 2048            # tokens per core (one batch)
H = 1024
NH_LOCAL = 4        # heads per core
HD = 64
CPC = NH_LOCAL * HD  # 256 channels per core
NF = H // 128       # feature chunks of the input dim
N_CORES = 8

_cache = threading.Lock()
_nc = None

LAST_RESULT = None  # BassKernelResults of the most recent run (for test.py)


DEBUG = bool(int(os.environ.get("MHA_DEBUG", "0")))
RECIP_MODE = os.environ.get("MHA_RECIP", "exact")


def _build_nc():
    nc = bacc.Bacc(None, target_bir_lowering=False, debug=False)

    xq_d = nc.dram_tensor("xq_t", [H, S], BF, kind="ExternalInput")
    xk_d = nc.dram_tensor("xk_t", [H, S], BF, kind="ExternalInput")
    xv_d = nc.dram_tensor("xv_t", [H, S], BF, kind="ExternalInput")
    wq_d = nc.dram_tensor("wq_t", [H, CPC], BF, kind="ExternalInput")
    wk_d = nc.dram_tensor("wk_t", [H, CPC], BF, kind="ExternalInput")
    wv_d = nc.dram_tensor("wv_t", [H, CPC], BF, kind="ExternalInput")
    bq_d = nc.dram_tensor("bq", [CPC, 1], F32, kind="ExternalInput")
    bk_d = nc.dram_tensor("bk", [CPC, 1], F32, kind="ExternalInput")
    bv_d = nc.dram_tensor("bv", [1, CPC], BF, kind="ExternalInput")
    wo_d = nc.dram_tensor("wo_t", [CPC, H], BF, kind="ExternalInput")
    y_d = nc.dram_tensor("y_t", [H, S], BF, kind="ExternalOutput")
    dbg_d = (
        nc.dram_tensor("dbg", [3, HD + 1, 512], F32, kind="ExternalOutput")
        if DEBUG
        else None
    )

    xq_ap = xq_d.rearrange("(nf p) s -> nf p s", p=128)
    xk_ap = xk_d.rearrange("(nf p) s -> nf p s", p=128)
    xv_ap = xv_d.rearrange("(nf p) s -> nf p s", p=128)
    y_ap = y_d.rearrange("(no p) s -> no p s", p=128)

    Exp = mybir.ActivationFunctionType.Exp
    Copy = mybir.ActivationFunctionType.Identity

    NSI = S // 128   # 16 key blocks
    NQS = S // 512   # 4 query slices

    with tile.TileContext(nc) as tc:
        with (
            tc.tile_pool(name="const", bufs=1) as const,
            tc.tile_pool(name="res", bufs=1) as res,
            tc.tile_pool(name="psum", bufs=1, space="PSUM") as psum,
            tc.tile_pool(name="epool", bufs=4) as epool,
            tc.tile_pool(name="npool", bufs=2) as npool,
            tc.tile_pool(name="onpool", bufs=2) as onpool,
            tc.tile_pool(name="ypool", bufs=3) as ypool,
        ):
            # --- constants / weights ---
            wq_sb = const.tile([128, NF, CPC], BF)
            wk_sb = const.tile([128, NF, CPC], BF)
            wv_sb = const.tile([128, NF, CPC], BF)
            wo_sb = const.tile([128, 2, NF, 128], BF)
            bq_sb = const.tile([128, 2], F32)
            bk_sb = const.tile([128, 2], F32)
            bv_sb = const.tile([1, CPC], BF)
            ones1 = const.tile([1, 128], BF)
            nc.sync.dma_start(wq_sb[:], wq_d.rearrange("(nf p) c -> p nf c", p=128))
            nc.sync.dma_start(wk_sb[:], wk_d.rearrange("(nf p) c -> p nf c", p=128))
            nc.sync.dma_start(wv_sb[:], wv_d.rearrange("(nf p) c -> p nf c", p=128))
            nc.sync.dma_start(
                wo_sb[:], wo_d.rearrange("(hp p) (no c) -> p hp no c", p=128, c=128)
            )
            nc.sync.dma_start(bq_sb[:], bq_d.rearrange("(hp p) one -> p (hp one)", p=128))
            nc.sync.dma_start(bk_sb[:], bk_d.rearrange("(hp p) one -> p (hp one)", p=128))
            nc.sync.dma_start(bv_sb[:], bv_d[:])
            nc.gpsimd.memset(ones1[:], 1.0)

            # --- residents ---
            # QT/KT per head-pair: partitions = 2 heads x 64 dims
            QT = [res.tile([128, S], BF, name=f"QT{hp}") for hp in range(2)]
            KT = [res.tile([128, S], BF, name=f"KT{hp}") for hp in range(2)]
            # V natural layout (keys on partitions), ones-augmented col 64
            V = res.tile([128, NH_LOCAL, NSI, HD + 1], BF)
            nc.gpsimd.memset(V[:, :, :, HD : HD + 1], 1.0)

            # --- projections ---
            with tc.tile_pool(name="xin", bufs=8) as xin:
                for name, x_ap, w_sb, b_sb, out_t in (
                    ("q", xq_ap, wq_sb, bq_sb, QT),
                    ("k", xk_ap, wk_sb, bk_sb, KT),
                ):
                    xt = []
                    for f in range(NF):
                        t = xin.tile([128, S], BF, tag=f"x{name}", name=f"x{name}{f}")
                        nc.sync.dma_start(t[:], x_ap[f])
                        xt.append(t)
                    for hp in range(2):
                        cs = slice(hp * 128, (hp + 1) * 128)
                        for sw in range(NQS):
                            ps = psum.tile([128, 512], F32, tag="pp", bufs=2,
                                           name=f"ps{name}{hp}{sw}")
                            for f in range(NF):
                                nc.tensor.matmul(
                                    ps[:],
                                    lhsT=w_sb[:, f, cs],
                                    rhs=xt[f][:, sw * 512 : (sw + 1) * 512],
                                    start=(f == 0),
                                    stop=(f == NF - 1),
                                )
                            nc.scalar.activation(
                                out_t[hp][:, sw * 512 : (sw + 1) * 512], ps[:], Copy,
                                bias=b_sb[:, hp : hp + 1],
                            )
                # V (natural layout, keys on partitions)
                xtv = []
                for f in range(NF):
                    t = xin.tile([128, S], BF, tag="xv", name=f"xv{f}")
                    nc.sync.dma_start(t[:], xv_ap[f])
                    xtv.append(t)
                for si in range(NSI):
                    psv = psum.tile([128, CPC], F32, tag="pp", bufs=2, name=f"psv{si}")
                    for f in range(NF):
                        nc.tensor.matmul(
                            psv[:],
                            lhsT=xtv[f][:, si * 128 : (si + 1) * 128],
                            rhs=wv_sb[:, f, :],
                            start=(f == 0),
                            stop=False,
                        )
                    nc.tensor.matmul(
                        psv[:], lhsT=ones1[:], rhs=bv_sb[:], start=False, stop=True
                    )
                    for h in range(NH_LOCAL):
                        nc.vector.tensor_copy(
                            V[:, h, si, 0:HD], psv[:, h * HD : (h + 1) * HD]
                        )

            # --- attention + output projection ---
            for qs in range(NQS):
                qsl = slice(qs * 512, (qs + 1) * 512)
                ON = []
                for hp in range(2):
                    h0, h1 = 2 * hp, 2 * hp + 1
                    po0 = psum.tile([HD + 1, 512], F32, tag="po0",
                                    name=f"po0_{qs}{hp}")
                    po1 = psum.tile([HD + 1, 512], F32, tag="po1",
                                    name=f"po1_{qs}{hp}")
                    for si in range(NSI):
                        ks = slice(si * 128, (si + 1) * 128)
                        ps = psum.tile([128, 1024], F32, tag="s", bufs=2,
                                       name=f"ps_{qs}{hp}{si}")
                        nc.tensor.matmul(
                            ps[:, 0:512],
                            lhsT=KT[hp][0:64, ks],
                            rhs=QT[hp][0:64, qsl],
                            tile_position=(0, 0),
                        )
                        nc.tensor.matmul(
                            ps[:, 512:1024],
                            lhsT=KT[hp][64:128, ks],
                            rhs=QT[hp][64:128, qsl],
                            tile_position=(64, 0),
                        )
                        e = epool.tile([128, 1024], BF, tag="e",
                                       name=f"e_{qs}{hp}{si}")
                        nc.scalar.activation(e[:], ps[:], Exp, scale=0.125)
                        nc.tensor.matmul(
                            po0[:], lhsT=V[:, h0, si, :], rhs=e[:, 0:512],
                            start=(si == 0), stop=(si == NSI - 1),
                        )
                        nc.tensor.matmul(
                            po1[:], lhsT=V[:, h1, si, :], rhs=e[:, 512:1024],
                            start=(si == 0), stop=(si == NSI - 1),
                        )
                    # norm: DVE-copy PSUM out (frees po banks early), then
                    # r = 1/sums (in-place on the sums row), DMA-shift to
                    # partition 0, broadcast, ON = O * r (all-SBUF muls)
                    so0 = npool.tile([HD + 1, 512], F32, tag="so0",
                                     name=f"so0_{qs}{hp}")
                    so1 = npool.tile([HD + 1, 512], F32, tag="so1",
                                     name=f"so1_{qs}{hp}")
                    nc.vector.tensor_copy(so0[:], po0[:])
                    nc.vector.tensor_copy(so1[:], po1[:])
                    if DEBUG and qs == 0 and hp == 0:
                        nc.sync.dma_start(dbg_d[0], so0[:])
                    rc0 = npool.tile([HD + 1, 512], F32, tag="rc0",
                                     name=f"rc0_{qs}{hp}")
                    rc1 = npool.tile([HD + 1, 512], F32, tag="rc1",
                                     name=f"rc1_{qs}{hp}")
                    if RECIP_MODE == "exact":
                        nc.vector.reciprocal(
                            rc0[HD : HD + 1, :], so0[HD : HD + 1, :]
                        )
                        nc.vector.reciprocal(
                            rc1[HD : HD + 1, :], so1[HD : HD + 1, :]
                        )
                    else:
                        nc.vector.reciprocal_approx_fast(
                            rc0[HD : HD + 1, :], so0[HD : HD + 1, :]
                        )
                        nc.vector.reciprocal_approx_fast(
                            rc1[HD : HD + 1, :], so1[HD : HD + 1, :]
                        )
                    ri0 = npool.tile([1, 512], F32, tag="ri0", name=f"ri0_{qs}{hp}")
                    ri1 = npool.tile([1, 512], F32, tag="ri1", name=f"ri1_{qs}{hp}")
                    nc.gpsimd.dma_start(ri0[:], rc0[HD : HD + 1, :])
                    nc.gpsimd.dma_start(ri1[:], rc1[HD : HD + 1, :])
                    rb0 = npool.tile([HD, 512], F32, tag="rb0", name=f"rb0_{qs}{hp}")
                    rb1 = npool.tile([HD, 512], F32, tag="rb1", name=f"rb1_{qs}{hp}")
                    nc.gpsimd.partition_broadcast(rb0[:], ri0[:])
                    nc.gpsimd.partition_broadcast(rb1[:], ri1[:])
                    if DEBUG and qs == 0 and hp == 0:
                        nc.sync.dma_start(dbg_d[1, 0:1, :], ri0[:])
                        nc.sync.dma_start(dbg_d[2, 0:HD, :], rb0[:])
                    on = onpool.tile([128, 512], BF, tag=f"on{hp}",
                                     name=f"on_{qs}{hp}")
                    on1 = onpool.tile([HD, 512], BF, tag="onx", name=f"onx_{qs}{hp}")
                    nc.vector.tensor_mul(on[0:HD, :], so0[0:HD, :], rb0[:])
                    nc.vector.tensor_mul(on1[:], so1[0:HD, :], rb1[:])
                    nc.gpsimd.dma_start(on[HD:128, :], on1[:])
                    ON.append(on)
                for oc in range(NF):
                    py = psum.tile([128, 512], F32, tag="pp", bufs=2, name=f"py_{qs}{oc}")
                    nc.tensor.matmul(
                        py[:], lhsT=wo_sb[:, 0, oc, :], rhs=ON[0][:],
                        start=True, stop=False,
                    )
                    nc.tensor.matmul(
                        py[:], lhsT=wo_sb[:, 1, oc, :], rhs=ON[1][:],
                        start=False, stop=True,
                    )
                    ysb = ypool.tile([128, 512], BF, tag="y", name=f"y_{qs}{oc}")
                    nc.vector.tensor_copy(ysb[:], py[:])
                    nc.sync.dma_start(y_ap[oc, :, qsl], ysb[:])
    nc.compile()
    return nc


def _get_nc():
    global _nc
    with _cache:
        if _nc is None:
            _nc = _build_nc()
        return _nc


def kernel(q, k, v, wq_w, wq_b, wk_w, wk_b, wv_w, wv_b, wo_w, wo_b):
    global LAST_RESULT
    nc = _get_nc()

    def xT(a, b):
        return np.ascontiguousarray(np.asarray(a)[b].astype(BF16).T)

    xs = {
        "xq_t": [xT(q, b) for b in range(B)],
        "xk_t": [xT(k, b) for b in range(B)],
        "xv_t": [xT(v, b) for b in range(B)],
    }
    wq_w = np.asarray(wq_w, dtype=np.float32)
    wk_w = np.asarray(wk_w, dtype=np.float32)
    wv_w = np.asarray(wv_w, dtype=np.float32)
    wo_w = np.asarray(wo_w, dtype=np.float32)

    in_maps = []
    for c in range(N_CORES):
        b, hg = c // 4, c % 4
        cs = slice(hg * CPC, (hg + 1) * CPC)
        in_maps.append({
            "xq_t": xs["xq_t"][b],
            "xk_t": xs["xk_t"][b],
            "xv_t": xs["xv_t"][b],
            "wq_t": np.ascontiguousarray(wq_w[cs, :].astype(BF16).T),
            "wk_t": np.ascontiguousarray(wk_w[cs, :].astype(BF16).T),
            "wv_t": np.ascontiguousarray(wv_w[cs, :].astype(BF16).T),
            "bq": np.asarray(wq_b, np.float32)[cs].reshape(CPC, 1),
            "bk": np.asarray(wk_b, np.float32)[cs].reshape(CPC, 1),
            "bv": np.asarray(wv_b, np.float32)[cs].astype(BF16).reshape(1, CPC),
            "wo_t": np.ascontiguousarray(wo_w[:, cs].astype(BF16).T),
        })

    res = run_bass_kernel_spmd(
        nc, in_maps, core_ids=list(range(N_CORES)),
        trace=bool(int(os.environ.get("MHA_TRACE", "0"))),
    )
    LAST_RESULT = res

    out = np.empty((B, S, H), dtype=np.float32)
    bias = np.asarray(wo_b, np.float64)[None, :]
    for b in range(B):
        y = res.results[4 * b]["y_t"].astype(np.float64)
        for hg in range(1, 4):
            y += res.results[4 * b + hg]["y_t"]
        out[b] = (y.T + bias).astype(np.float32)
    return out


# revision 14
# speedup vs baseline: 1.9525x; 1.0622x over previous
"""Multi-head attention (B=2, S=2048, H=1024, 16 heads) on 8 trn2 NeuronCores.

Sharding: 2-way batch x 4-way head tensor parallel. Core c owns batch c//4 and
heads 4*(c%4) .. 4*(c%4)+4 (256 channels of the QKV projections, 256 input
channels of the output projection). Each core consumes its batch's activations
(transposed, bf16) and returns a bf16 partial of the wo projection; the host
sums the 4 partials per batch and adds the bias.

Device dataflow per core (matmuls bf16, f32 PSUM):
  QT/KT[c,s]: transposed projections per head-pair hp (c = 2 heads x 64 dims)
  V[s, h, si, d+1]: natural layout, ones-augmented col for softmax sums
  attention, software-pipelined across (qs, hp) unit boundaries so the PE
  queue never waits on the scalar-engine exp tail:
    scores(u, si): ps[128k, 2x512q] = KT_hp^T . QT_hp  (two heads, PE quads)
    e(u, si) = exp(ps/8)   (one ACT op per key block, both heads)
    attnV(u, si) lags LAG steps: po_h[65, 512] += [V_h|1]^T e_h
  norm: r = 1/sums via DVE reciprocal, partition-broadcast, ON = O * r
  wo:   py[oc, 512q] = sum_hp woT_hp,oc . ON_hp ; bf16 -> y_t
"""

import os
import threading

import numpy as np
import ml_dtypes

import concourse.bass as bass
import concourse.mybir as mybir
import concourse.tile as tile
from concourse import bacc
from concourse.bass_utils import run_bass_kernel_spmd

BF16 = ml_dtypes.bfloat16
F32 = mybir.dt.float32
BF = mybir.dt.bfloat16

B = 2
S = 2048            # tokens per core (one batch)
H = 1024
NH_LOCAL = 4        # heads per core
HD = 64
CPC = NH_LOCAL * HD  # 256 channels per core
NF = H // 128       # feature chunks of the input dim
N_CORES = 8
LAG = 3             # attnV lag (in key-block steps) behind scores/exp

_cache = threading.Lock()
_nc = None

LAST_RESULT = None  # BassKernelResults of the most recent run (for test.py)

DEBUG = bool(int(os.environ.get("MHA_DEBUG", "0")))


def _build_nc():
    nc = bacc.Bacc(None, target_bir_lowering=False, debug=False)

    xq_d = nc.dram_tensor("xq_t", [H, S], BF, kind="ExternalInput")
    xk_d = nc.dram_tensor("xk_t", [H, S], BF, kind="ExternalInput")
    xv_d = nc.dram_tensor("xv_t", [H, S], BF, kind="ExternalInput")
    wq_d = nc.dram_tensor("wq_t", [H, CPC], BF, kind="ExternalInput")
    wk_d = nc.dram_tensor("wk_t", [H, CPC], BF, kind="ExternalInput")
    wv_d = nc.dram_tensor("wv_t", [H, CPC], BF, kind="ExternalInput")
    bq_d = nc.dram_tensor("bq", [CPC, 1], F32, kind="ExternalInput")
    bk_d = nc.dram_tensor("bk", [CPC, 1], F32, kind="ExternalInput")
    bv_d = nc.dram_tensor("bv", [1, CPC], BF, kind="ExternalInput")
    wo_d = nc.dram_tensor("wo_t", [CPC, H], BF, kind="ExternalInput")
    y_d = nc.dram_tensor("y_t", [H, S], BF, kind="ExternalOutput")

    xq_ap = xq_d.rearrange("(nf p) s -> nf p s", p=128)
    xk_ap = xk_d.rearrange("(nf p) s -> nf p s", p=128)
    xv_ap = xv_d.rearrange("(nf p) s -> nf p s", p=128)
    wq_ap = wq_d.rearrange("(nf p) c -> nf p c", p=128)
    wk_ap = wk_d.rearrange("(nf p) c -> nf p c", p=128)
    wv_ap = wv_d.rearrange("(nf p) c -> nf p c", p=128)
    y_ap = y_d.rearrange("(no p) s -> no p s", p=128)

    Exp = mybir.ActivationFunctionType.Exp
    Copy = mybir.ActivationFunctionType.Identity

    NSI = S // 128   # 16 key blocks
    NQS = S // 512   # 4 query slices

    with tile.TileContext(nc) as tc:
        with (
            tc.tile_pool(name="const", bufs=1) as const,
            tc.tile_pool(name="res", bufs=1) as res,
            tc.tile_pool(name="psum", bufs=1, space="PSUM") as psum,
            tc.tile_pool(name="epool", bufs=6) as epool,
            tc.tile_pool(name="npool", bufs=2) as npool,
            tc.tile_pool(name="onpool", bufs=2) as onpool,
            tc.tile_pool(name="ypool", bufs=3) as ypool,
        ):
            # --- constants / weights (per-chunk so compute starts early) ---
            wq_sb = const.tile([128, NF, CPC], BF)
            wk_sb = const.tile([128, NF, CPC], BF)
            wv_sb = const.tile([128, NF, CPC], BF)
            wo_sb = const.tile([128, 2, NF, 128], BF)
            bq_sb = const.tile([128, 2], F32)
            bk_sb = const.tile([128, 2], F32)
            bv_sb = const.tile([1, CPC], BF)
            ones1 = const.tile([1, 128], BF)
            for f in range(NF):
                nc.sync.dma_start(wq_sb[:, f, :], wq_ap[f])
                nc.sync.dma_start(wk_sb[:, f, :], wk_ap[f])
                nc.sync.dma_start(wv_sb[:, f, :], wv_ap[f])
            nc.sync.dma_start(
                wo_sb[:], wo_d.rearrange("(hp p) (no c) -> p hp no c", p=128, c=128)
            )
            nc.sync.dma_start(bq_sb[:], bq_d.rearrange("(hp p) one -> p (hp one)", p=128))
            nc.sync.dma_start(bk_sb[:], bk_d.rearrange("(hp p) one -> p (hp one)", p=128))
            nc.sync.dma_start(bv_sb[:], bv_d[:])
            nc.gpsimd.memset(ones1[:], 1.0)

            # --- residents ---
            QT = [res.tile([128, S], BF, name=f"QT{hp}") for hp in range(2)]
            KT = [res.tile([128, S], BF, name=f"KT{hp}") for hp in range(2)]
            V = res.tile([128, NH_LOCAL, NSI, HD + 1], BF)
            nc.gpsimd.memset(V[:, :, :, HD : HD + 1], 1.0)

            # --- projections (chunk-streaming: f outer, psum groups live) ---
            with tc.tile_pool(name="xin", bufs=8) as xin:
                for name, x_ap, w_sb, b_sb, out_t in (
                    ("q", xq_ap, wq_sb, bq_sb, QT),
                    ("k", xk_ap, wk_sb, bk_sb, KT),
                ):
                    xt = []
                    for f in range(NF):
                        t = xin.tile([128, S], BF, tag=f"x{name}", name=f"x{name}{f}")
                        nc.sync.dma_start(t[:], x_ap[f])
                        xt.append(t)
                    for half in range(2):
                        pst = {}
                        for hp in range(2):
                            for swh in range(2):
                                tag = ("s", "po0", "po1", "s")[2 * hp + swh]
                                pst[hp, swh] = psum.tile(
                                    [128, 512], F32, tag=tag, bufs=2,
                                    name=f"ps{name}{half}{hp}{swh}",
                                )
                        for f in range(NF):
                            for hp in range(2):
                                cs = slice(hp * 128, (hp + 1) * 128)
                                for swh in range(2):
                                    sw = half * 2 + swh
                                    nc.tensor.matmul(
                                        pst[hp, swh][:],
                                        lhsT=w_sb[:, f, cs],
                                        rhs=xt[f][:, sw * 512 : (sw + 1) * 512],
                                        start=(f == 0),
                                        stop=(f == NF - 1),
                                    )
                        for hp in range(2):
                            for swh in range(2):
                                sw = half * 2 + swh
                                nc.scalar.activation(
                                    out_t[hp][:, sw * 512 : (sw + 1) * 512],
                                    pst[hp, swh][:], Copy,
                                    bias=b_sb[:, hp : hp + 1],
                                )
                # V (natural layout, keys on partitions)
                xtv = []
                for f in range(NF):
                    t = xin.tile([128, S], BF, tag="xv", name=f"xv{f}")
                    nc.sync.dma_start(t[:], xv_ap[f])
                    xtv.append(t)
                for si in range(NSI):
                    psv = psum.tile([128, CPC], F32, tag="po0", bufs=2,
                                    name=f"psv{si}")
                    for f in range(NF):
                        nc.tensor.matmul(
                            psv[:],
                            lhsT=xtv[f][:, si * 128 : (si + 1) * 128],
                            rhs=wv_sb[:, f, :],
                            start=(f == 0),
                            stop=False,
                        )
                    nc.tensor.matmul(
                        psv[:], lhsT=ones1[:], rhs=bv_sb[:], start=False, stop=True
                    )
                    for h in range(NH_LOCAL):
                        if h < 2:
                            nc.vector.tensor_copy(
                                V[:, h, si, 0:HD], psv[:, h * HD : (h + 1) * HD]
                            )
                        else:
                            nc.scalar.copy(
                                V[:, h, si, 0:HD], psv[:, h * HD : (h + 1) * HD]
                            )

            # --- attention, software-pipelined across unit boundaries ---
            units = [(qs, hp) for qs in range(NQS) for hp in range(2)]
            nsteps = len(units) * NSI
            po = {}   # (u_idx) -> (po0, po1)
            e_t = {}  # step -> e tile
            ON = {}   # qs -> [on_hp0, on_hp1]

            def emit_scores(t):
                u, si = divmod(t, NSI)
                qs, hp = units[u]
                qsl = slice(qs * 512, (qs + 1) * 512)
                ks = slice(si * 128, (si + 1) * 128)
                if si == 0:
                    po[u] = (
                        psum.tile([HD + 1, 512], F32, tag="po0", bufs=2,
                                  name=f"po0_{qs}{hp}"),
                        psum.tile([HD + 1, 512], F32, tag="po1", bufs=2,
                                  name=f"po1_{qs}{hp}"),
                    )
                ps = psum.tile([128, 1024], F32, tag="s", bufs=2,
                               name=f"ps_{qs}{hp}{si}")
                nc.tensor.matmul(
                    ps[:, 0:512],
                    lhsT=KT[hp][0:64, ks],
                    rhs=QT[hp][0:64, qsl],
                    tile_position=(0, 0),
                )
                nc.tensor.matmul(
                    ps[:, 512:1024],
                    lhsT=KT[hp][64:128, ks],
                    rhs=QT[hp][64:128, qsl],
                    tile_position=(64, 0),
                )
                e = epool.tile([128, 1024], BF, tag="e", name=f"e_{qs}{hp}{si}")
                nc.scalar.activation(e[:], ps[:], Exp, scale=0.125)
                e_t[t] = e

            def emit_attnv(t):
                u, si = divmod(t, NSI)
                qs, hp = units[u]
                h0, h1 = 2 * hp, 2 * hp + 1
                po0, po1 = po[u]
                e = e_t.pop(t)
                nc.tensor.matmul(
                    po0[:], lhsT=V[:, h0, si, :], rhs=e[:, 0:512],
                    start=(si == 0), stop=(si == NSI - 1),
                )
                nc.tensor.matmul(
                    po1[:], lhsT=V[:, h1, si, :], rhs=e[:, 512:1024],
                    start=(si == 0), stop=(si == NSI - 1),
                )
                if si == NSI - 1:
                    emit_norm(u)
                    if hp == 1:
                        emit_wo(qs)

            def emit_norm(u):
                qs, hp = units[u]
                po0, po1 = po.pop(u)
                # r = 1/sums into row 64 of an SBUF tile, shift to partition 0,
                # broadcast, ON = O * r (O read straight from PSUM)
                rc0 = npool.tile([HD + 1, 512], F32, tag="rc0", name=f"rc0_{qs}{hp}")
                rc1 = npool.tile([HD + 1, 512], F32, tag="rc1", name=f"rc1_{qs}{hp}")
                nc.vector.reciprocal(rc0[HD : HD + 1, :], po0[HD : HD + 1, :])
                nc.vector.reciprocal(rc1[HD : HD + 1, :], po1[HD : HD + 1, :])
                ri0 = npool.tile([1, 512], F32, tag="ri0", name=f"ri0_{qs}{hp}")
                ri1 = npool.tile([1, 512], F32, tag="ri1", name=f"ri1_{qs}{hp}")
                nc.gpsimd.dma_start(ri0[:], rc0[HD : HD + 1, :])
                nc.gpsimd.dma_start(ri1[:], rc1[HD : HD + 1, :])
                rb0 = npool.tile([HD, 512], F32, tag="rb0", name=f"rb0_{qs}{hp}")
                rb1 = npool.tile([HD, 512], F32, tag="rb1", name=f"rb1_{qs}{hp}")
                nc.gpsimd.partition_broadcast(rb0[:], ri0[:])
                nc.gpsimd.partition_broadcast(rb1[:], ri1[:])
                on = onpool.tile([128, 512], BF, tag=f"on{hp}", name=f"on_{qs}{hp}")
                on1 = onpool.tile([HD, 512], BF, tag="onx", name=f"onx_{qs}{hp}")
                nc.vector.tensor_mul(on[0:HD, :], po0[0:HD, :], rb0[:])
                nc.vector.tensor_mul(on1[:], po1[0:HD, :], rb1[:])
                nc.gpsimd.dma_start(on[HD:128, :], on1[:])
                ON.setdefault(qs, {})[hp] = on

            def emit_wo(qs):
                qsl = slice(qs * 512, (qs + 1) * 512)
                on_by_hp = ON.pop(qs)
                for oc in range(NF):
                    py = psum.tile([128, 512], F32, tag="s", bufs=2,
                                   name=f"py_{qs}{oc}")
                    nc.tensor.matmul(
                        py[:], lhsT=wo_sb[:, 0, oc, :], rhs=on_by_hp[0][:],
                        start=True, stop=False,
                    )
                    nc.tensor.matmul(
                        py[:], lhsT=wo_sb[:, 1, oc, :], rhs=on_by_hp[1][:],
                        start=False, stop=True,
                    )
                    ysb = ypool.tile([128, 512], BF, tag="y", name=f"y_{qs}{oc}")
                    nc.vector.tensor_copy(ysb[:], py[:])
                    nc.sync.dma_start(y_ap[oc, :, qsl], ysb[:])

            for t in range(nsteps + LAG):
                if t < nsteps:
                    emit_scores(t)
                if t >= LAG:
                    emit_attnv(t - LAG)
    nc.compile()
    return nc


def _get_nc():
    global _nc
    with _cache:
        if _nc is None:
            _nc = _build_nc()
        return _nc


def kernel(q, k, v, wq_w, wq_b, wk_w, wk_b, wv_w, wv_b, wo_w, wo_b):
    global LAST_RESULT
    nc = _get_nc()

    def xT(a, b):
        return np.ascontiguousarray(np.asarray(a)[b].astype(BF16).T)

    xs = {
        "xq_t": [xT(q, b) for b in range(B)],
        "xk_t": [xT(k, b) for b in range(B)],
        "xv_t": [xT(v, b) for b in range(B)],
    }
    wq_w = np.asarray(wq_w, dtype=np.float32)
    wk_w = np.asarray(wk_w, dtype=np.float32)
    wv_w = np.asarray(wv_w, dtype=np.float32)
    wo_w = np.asarray(wo_w, dtype=np.float32)

    in_maps = []
    for c in range(N_CORES):
        b, hg = c // 4, c % 4
        cs = slice(hg * CPC, (hg + 1) * CPC)
        in_maps.append({
            "xq_t": xs["xq_t"][b],
            "xk_t": xs["xk_t"][b],
            "xv_t": xs["xv_t"][b],
            "wq_t": np.ascontiguousarray(wq_w[cs, :].astype(BF16).T),
            "wk_t": np.ascontiguousarray(wk_w[cs, :].astype(BF16).T),
            "wv_t": np.ascontiguousarray(wv_w[cs, :].astype(BF16).T),
            "bq": np.asarray(wq_b, np.float32)[cs].reshape(CPC, 1),
            "bk": np.asarray(wk_b, np.float32)[cs].reshape(CPC, 1),
            "bv": np.asarray(wv_b, np.float32)[cs].astype(BF16).reshape(1, CPC),
            "wo_t": np.ascontiguousarray(wo_w[:, cs].astype(BF16).T),
        })

    res = run_bass_kernel_spmd(
        nc, in_maps, core_ids=list(range(N_CORES)),
        trace=bool(int(os.environ.get("MHA_TRACE", "0"))),
    )
    LAST_RESULT = res

    out = np.empty((B, S, H), dtype=np.float32)
    bias = np.asarray(wo_b, np.float64)[None, :]
    for b in range(B):
        y = res.results[4 * b]["y_t"].astype(np.float64)
        for hg in range(1, 4):
            y += res.results[4 * b + hg]["y_t"]
        out[b] = (y.T + bias).astype(np.float32)
    return out


# revision 19
# speedup vs baseline: 2.1831x; 1.1181x over previous
"""Multi-head attention (B=2, S=2048, H=1024, 16 heads) on 8 trn2 NeuronCores.

Sharding: 2-way batch x 4-way head tensor parallel. Core c owns batch c//4 and
heads 4*(c%4) .. 4*(c%4)+4 (256 channels of the QKV projections, 256 input
channels of the output projection). Each core consumes its batch's activations
(transposed, bf16) and returns a bf16 partial of the wo projection; the host
sums the 4 partials per batch and adds the bias.

Device dataflow per core (matmuls bf16, f32 PSUM):
  QT/KT[c,s]: transposed projections per head-pair hp (c = 2 heads x 64 dims)
  V[s, h, si, d+1]: natural layout, ones-augmented col for softmax sums
  attention, software-pipelined across (qs, hp) unit boundaries so the PE
  queue never waits on the scalar-engine exp tail:
    scores(u, si): ps[128k, 2x512q] = KT_hp^T . QT_hp  (two heads, PE quads)
    e(u, si) = exp(ps/8)   (one ACT op per key block, both heads)
    attnV(u, si) lags LAG steps: po_h[65, 512] += [V_h|1]^T e_h
  norm: r = 1/sums via DVE reciprocal, partition-broadcast, ON = O * r
  wo:   py[oc, 512q] = sum_hp woT_hp,oc . ON_hp ; bf16 -> y_t
"""

import os
import threading

import numpy as np
import ml_dtypes

import concourse.bass as bass
import concourse.mybir as mybir
import concourse.tile as tile
from concourse import bacc
from concourse.bass_utils import run_bass_kernel_spmd

BF16 = ml_dtypes.bfloat16
F32 = mybir.dt.float32
BF = mybir.dt.bfloat16

B = 2
S = 2048            # tokens per core (one batch)
H = 1024
NH_LOCAL = 4        # heads per core
HD = 64
CPC = NH_LOCAL * HD  # 256 channels per core
NF = H // 128       # feature chunks of the input dim
N_CORES = 8
LAG = 3             # attnV lag (in key-block steps) behind scores/exp
LAG_WO = 8          # wo deferral (steps) to hide the softmax-norm latency

_cache = threading.Lock()
_nc = None

LAST_RESULT = None  # BassKernelResults of the most recent run (for test.py)

DEBUG = bool(int(os.environ.get("MHA_DEBUG", "0")))


def _build_nc():
    nc = bacc.Bacc(None, target_bir_lowering=False, debug=False)

    xq_d = nc.dram_tensor("xq_t", [H, S], BF, kind="ExternalInput")
    xk_d = nc.dram_tensor("xk_t", [H, S], BF, kind="ExternalInput")
    xv_d = nc.dram_tensor("xv_t", [H, S], BF, kind="ExternalInput")
    wq_d = nc.dram_tensor("wq_t", [H, CPC], BF, kind="ExternalInput")
    wk_d = nc.dram_tensor("wk_t", [H, CPC], BF, kind="ExternalInput")
    wv_d = nc.dram_tensor("wv_t", [H, CPC], BF, kind="ExternalInput")
    bq_d = nc.dram_tensor("bq", [CPC, 1], F32, kind="ExternalInput")
    bk_d = nc.dram_tensor("bk", [CPC, 1], F32, kind="ExternalInput")
    bv_d = nc.dram_tensor("bv", [1, CPC], BF, kind="ExternalInput")
    wo_d = nc.dram_tensor("wo_t", [CPC, H], BF, kind="ExternalInput")
    y_d = nc.dram_tensor("y_t", [H, S], BF, kind="ExternalOutput")

    xq_ap = xq_d.rearrange("(nf p) s -> nf p s", p=128)
    xk_ap = xk_d.rearrange("(nf p) s -> nf p s", p=128)
    xv_ap = xv_d.rearrange("(nf p) s -> nf p s", p=128)
    wq_ap = wq_d.rearrange("(nf p) c -> nf p c", p=128)
    wk_ap = wk_d.rearrange("(nf p) c -> nf p c", p=128)
    wv_ap = wv_d.rearrange("(nf p) c -> nf p c", p=128)
    y_ap = y_d.rearrange("(no p) s -> no p s", p=128)

    Exp = mybir.ActivationFunctionType.Exp
    Copy = mybir.ActivationFunctionType.Identity

    NSI = S // 128   # 16 key blocks
    NQS = S // 512   # 4 query slices

    with tile.TileContext(nc) as tc:
        with (
            tc.tile_pool(name="const", bufs=1) as const,
            tc.tile_pool(name="res", bufs=1) as res,
            tc.tile_pool(name="psum", bufs=1, space="PSUM") as psum,
            tc.tile_pool(name="epool", bufs=6) as epool,
            tc.tile_pool(name="npool", bufs=2) as npool,
            tc.tile_pool(name="onpool", bufs=2) as onpool,
            tc.tile_pool(name="ypool", bufs=3) as ypool,
        ):
            # --- constants / weights (per-chunk, spread over issue queues so
            # the first projection matmul can start as early as possible) ---
            wq_sb = const.tile([128, NF, CPC], BF)
            wk_sb = const.tile([128, NF, CPC], BF)
            wv_sb = const.tile([128, NF, CPC], BF)
            wo_sb = const.tile([128, 2, NF, 128], BF)
            bq_sb = const.tile([128, 2], F32)
            bk_sb = const.tile([128, 2], F32)
            bv_sb = const.tile([1, CPC], BF)
            ones1 = const.tile([1, 128], BF)
            for f in range(NF):
                nc.scalar.dma_start(wq_sb[:, f, :], wq_ap[f])
            nc.scalar.dma_start(
                bq_sb[:], bq_d.rearrange("(hp p) one -> p (hp one)", p=128)
            )
            for f in range(NF):
                nc.scalar.dma_start(wk_sb[:, f, :], wk_ap[f])
            nc.scalar.dma_start(
                bk_sb[:], bk_d.rearrange("(hp p) one -> p (hp one)", p=128)
            )
            for f in range(NF):
                nc.gpsimd.dma_start(wv_sb[:, f, :], wv_ap[f])
            nc.gpsimd.dma_start(bv_sb[:], bv_d[:])
            nc.scalar.dma_start(
                wo_sb[:], wo_d.rearrange("(hp p) (no c) -> p hp no c", p=128, c=128)
            )
            nc.gpsimd.memset(ones1[:], 1.0)

            # --- residents ---
            QT = [res.tile([128, S], BF, name=f"QT{hp}") for hp in range(2)]
            KT = [res.tile([128, S], BF, name=f"KT{hp}") for hp in range(2)]
            V = res.tile([128, NH_LOCAL, NSI, HD + 1], BF)
            nc.gpsimd.memset(V[:, :, :, HD : HD + 1], 1.0)

            # --- projections (chunk-streaming: f outer, psum groups live) ---
            with tc.tile_pool(name="xin", bufs=8) as xin:
                for name, x_ap, w_sb, b_sb, out_t in (
                    ("q", xq_ap, wq_sb, bq_sb, QT),
                    ("k", xk_ap, wk_sb, bk_sb, KT),
                ):
                    xt = []
                    for f in range(NF):
                        t = xin.tile([128, S], BF, tag=f"x{name}", name=f"x{name}{f}")
                        nc.sync.dma_start(t[:], x_ap[f])
                        xt.append(t)
                    for half in range(2):
                        pst = {}
                        for hp in range(2):
                            for swh in range(2):
                                tag = ("s", "po0", "po1", "s")[2 * hp + swh]
                                pst[hp, swh] = psum.tile(
                                    [128, 512], F32, tag=tag, bufs=2,
                                    name=f"ps{name}{half}{hp}{swh}",
                                )
                        for f in range(NF):
                            for hp in range(2):
                                cs = slice(hp * 128, (hp + 1) * 128)
                                for swh in range(2):
                                    sw = half * 2 + swh
                                    nc.tensor.matmul(
                                        pst[hp, swh][:],
                                        lhsT=w_sb[:, f, cs],
                                        rhs=xt[f][:, sw * 512 : (sw + 1) * 512],
                                        start=(f == 0),
                                        stop=(f == NF - 1),
                                    )
                        for hp in range(2):
                            for swh in range(2):
                                sw = half * 2 + swh
                                nc.scalar.activation(
                                    out_t[hp][:, sw * 512 : (sw + 1) * 512],
                                    pst[hp, swh][:], Copy,
                                    bias=b_sb[:, hp : hp + 1],
                                )
                # V (natural layout, keys on partitions)
                xtv = []
                for f in range(NF):
                    t = xin.tile([128, S], BF, tag="xv", name=f"xv{f}")
                    nc.sync.dma_start(t[:], xv_ap[f])
                    xtv.append(t)
                for si in range(NSI):
                    psv = psum.tile([128, CPC], F32, tag="po0", bufs=2,
                                    name=f"psv{si}")
                    for f in range(NF):
                        nc.tensor.matmul(
                            psv[:],
                            lhsT=xtv[f][:, si * 128 : (si + 1) * 128],
                            rhs=wv_sb[:, f, :],
                            start=(f == 0),
                            stop=False,
                        )
                    nc.tensor.matmul(
                        psv[:], lhsT=ones1[:], rhs=bv_sb[:], start=False, stop=True
                    )
                    for h in range(NH_LOCAL):
                        if h < 2:
                            nc.vector.tensor_copy(
                                V[:, h, si, 0:HD], psv[:, h * HD : (h + 1) * HD]
                            )
                        else:
                            nc.scalar.copy(
                                V[:, h, si, 0:HD], psv[:, h * HD : (h + 1) * HD]
                            )

            # --- attention, software-pipelined across unit boundaries ---
            units = [(qs, hp) for qs in range(NQS) for hp in range(2)]
            nsteps = len(units) * NSI
            po = {}   # (u_idx) -> (po0, po1)
            e_t = {}  # step -> e tile
            ON = {}   # qs -> [on_hp0, on_hp1]

            def emit_scores(t):
                u, si = divmod(t, NSI)
                qs, hp = units[u]
                qsl = slice(qs * 512, (qs + 1) * 512)
                ks = slice(si * 128, (si + 1) * 128)
                if si == 0:
                    po[u] = (
                        psum.tile([HD + 1, 512], F32, tag="po0", bufs=2,
                                  name=f"po0_{qs}{hp}"),
                        psum.tile([HD + 1, 512], F32, tag="po1", bufs=2,
                                  name=f"po1_{qs}{hp}"),
                    )
                ps = psum.tile([128, 1024], F32, tag="s", bufs=2,
                               name=f"ps_{qs}{hp}{si}")
                nc.tensor.matmul(
                    ps[:, 0:512],
                    lhsT=KT[hp][0:64, ks],
                    rhs=QT[hp][0:64, qsl],
                    tile_position=(0, 0),
                )
                nc.tensor.matmul(
                    ps[:, 512:1024],
                    lhsT=KT[hp][64:128, ks],
                    rhs=QT[hp][64:128, qsl],
                    tile_position=(64, 0),
                )
                e = epool.tile([128, 1024], BF, tag="e", name=f"e_{qs}{hp}{si}")
                nc.scalar.activation(e[:], ps[:], Exp, scale=0.125)
                e_t[t] = e

            def emit_attnv(t):
                u, si = divmod(t, NSI)
                qs, hp = units[u]
                h0, h1 = 2 * hp, 2 * hp + 1
                po0, po1 = po[u]
                e = e_t.pop(t)
                nc.tensor.matmul(
                    po0[:], lhsT=V[:, h0, si, :], rhs=e[:, 0:512],
                    start=(si == 0), stop=(si == NSI - 1),
                )
                nc.tensor.matmul(
                    po1[:], lhsT=V[:, h1, si, :], rhs=e[:, 512:1024],
                    start=(si == 0), stop=(si == NSI - 1),
                )
                if si == NSI - 1:
                    emit_norm(u)
                    if hp == 1:
                        # defer the wo matmuls LAG_WO steps so the norm chain
                        # (DVE reciprocal + broadcast) completes in the shadow
                        # of the next unit's scores/attnV work
                        pending_wo.append([qs, t + LAG_WO])

            def emit_norm(u):
                qs, hp = units[u]
                po0, po1 = po.pop(u)
                # r = 1/sums into row 64 of an SBUF tile, shift to partition 0,
                # broadcast, ON = O * r (O read straight from PSUM)
                rc0 = npool.tile([HD + 1, 512], F32, tag="rc0", name=f"rc0_{qs}{hp}")
                rc1 = npool.tile([HD + 1, 512], F32, tag="rc1", name=f"rc1_{qs}{hp}")
                nc.vector.reciprocal(rc0[HD : HD + 1, :], po0[HD : HD + 1, :])
                nc.vector.reciprocal(rc1[HD : HD + 1, :], po1[HD : HD + 1, :])
                ri0 = npool.tile([1, 512], F32, tag="ri0", name=f"ri0_{qs}{hp}")
                ri1 = npool.tile([1, 512], F32, tag="ri1", name=f"ri1_{qs}{hp}")
                nc.gpsimd.dma_start(ri0[:], rc0[HD : HD + 1, :])
                nc.gpsimd.dma_start(ri1[:], rc1[HD : HD + 1, :])
                rb0 = npool.tile([HD, 512], F32, tag="rb0", name=f"rb0_{qs}{hp}")
                rb1 = npool.tile([HD, 512], F32, tag="rb1", name=f"rb1_{qs}{hp}")
                nc.gpsimd.partition_broadcast(rb0[:], ri0[:])
                nc.gpsimd.partition_broadcast(rb1[:], ri1[:])
                on = onpool.tile([128, 512], BF, tag=f"on{hp}", name=f"on_{qs}{hp}")
                on1 = onpool.tile([HD, 512], BF, tag="onx", name=f"onx_{qs}{hp}")
                nc.vector.tensor_mul(on[0:HD, :], po0[0:HD, :], rb0[:])
                nc.vector.tensor_mul(on1[:], po1[0:HD, :], rb1[:])
                nc.gpsimd.dma_start(on[HD:128, :], on1[:])
                ON.setdefault(qs, {})[hp] = on

            def emit_wo(qs):
                qsl = slice(qs * 512, (qs + 1) * 512)
                on_by_hp = ON.pop(qs)
                for oc in range(NF):
                    py = psum.tile([128, 512], F32, tag="s", bufs=2,
                                   name=f"py_{qs}{oc}")
                    nc.tensor.matmul(
                        py[:], lhsT=wo_sb[:, 0, oc, :], rhs=on_by_hp[0][:],
                        start=True, stop=False,
                    )
                    nc.tensor.matmul(
                        py[:], lhsT=wo_sb[:, 1, oc, :], rhs=on_by_hp[1][:],
                        start=False, stop=True,
                    )
                    ysb = ypool.tile([128, 512], BF, tag="y", name=f"y_{qs}{oc}")
                    nc.vector.tensor_copy(ysb[:], py[:])
                    nc.sync.dma_start(y_ap[oc, :, qsl], ysb[:])

            pending_wo = []
            for t in range(nsteps + LAG):
                if t < nsteps:
                    emit_scores(t)
                if t >= LAG:
                    emit_attnv(t - LAG)
                while pending_wo and pending_wo[0][1] <= t - LAG:
                    emit_wo(pending_wo.pop(0)[0])
            while pending_wo:
                emit_wo(pending_wo.pop(0)[0])
    nc.compile()
    return nc


def _get_nc():
    global _nc
    with _cache:
        if _nc is None:
            _nc = _build_nc()
        return _nc


def kernel(q, k, v, wq_w, wq_b, wk_w, wk_b, wv_w, wv_b, wo_w, wo_b):
    global LAST_RESULT
    nc = _get_nc()

    def xT(a, b):
        return np.ascontiguousarray(np.asarray(a)[b].astype(BF16).T)

    xs = {
        "xq_t": [xT(q, b) for b in range(B)],
        "xk_t": [xT(k, b) for b in range(B)],
        "xv_t": [xT(v, b) for b in range(B)],
    }
    wq_w = np.asarray(wq_w, dtype=np.float32)
    wk_w = np.asarray(wk_w, dtype=np.float32)
    wv_w = np.asarray(wv_w, dtype=np.float32)
    wo_w = np.asarray(wo_w, dtype=np.float32)

    in_maps = []
    for c in range(N_CORES):
        b, hg = c // 4, c % 4
        cs = slice(hg * CPC, (hg + 1) * CPC)
        in_maps.append({
            "xq_t": xs["xq_t"][b],
            "xk_t": xs["xk_t"][b],
            "xv_t": xs["xv_t"][b],
            "wq_t": np.ascontiguousarray(wq_w[cs, :].astype(BF16).T),
            "wk_t": np.ascontiguousarray(wk_w[cs, :].astype(BF16).T),
            "wv_t": np.ascontiguousarray(wv_w[cs, :].astype(BF16).T),
            "bq": np.asarray(wq_b, np.float32)[cs].reshape(CPC, 1),
            "bk": np.asarray(wk_b, np.float32)[cs].reshape(CPC, 1),
            "bv": np.asarray(wv_b, np.float32)[cs].astype(BF16).reshape(1, CPC),
            "wo_t": np.ascontiguousarray(wo_w[:, cs].astype(BF16).T),
        })

    res = run_bass_kernel_spmd(
        nc, in_maps, core_ids=list(range(N_CORES)),
        trace=bool(int(os.environ.get("MHA_TRACE", "0"))),
    )
    LAST_RESULT = res

    out = np.empty((B, S, H), dtype=np.float32)
    bias = np.asarray(wo_b, np.float64)[None, :]
    for b in range(B):
        y = res.results[4 * b]["y_t"].astype(np.float64)
        for hg in range(1, 4):
            y += res.results[4 * b + hg]["y_t"]
        out[b] = (y.T + bias).astype(np.float32)
    return out
